# revision 15
# baseline (speedup 1.0000x reference)
"""STAR/SMPL-style LBS forward, data-parallel over batch x vertex groups on 8 trn2 cores.

Sharding: 2 batch groups x 4 vertex groups = 8 cores.
Per core: B_sh=256 batches (2 partition-tiles of 128), NSH=1728 vertices.

Host-side folds (numpy, validated vs reference to ~3e-7 rel):
  - shapedirs^T augmented with v_template row -> mm1 gives v_shaped directly
  - J_regressor folded into betas-space (JS block) -> rest joints J from mm1
  - parent-differences folded (JSL block) -> chain-local translations from mm1
  - trans folded into the kinematic-chain root translation
  - the reference's second kinematic chain == the first chain pre-correction,
    so J_transformed is read from the single chain before the -G@rest fix.
"""
import sys

for _p in ("/opt/trn_rl_repo",):
    if _p not in sys.path:
        sys.path.insert(0, _p)

import numpy as np

import concourse.bacc as bacc
import concourse.bass as bass
import concourse.mybir as mybir
import concourse.tile as tile
from concourse import bass_utils
from concourse.masks import make_identity

F32 = mybir.dt.float32

B = 512
NJ = 24
NV = 6890
K = 300
KA = K + 1          # betas augmented with ones row
F = 93              # pose_feat width
PARENT = [0, 0, 0, 1, 2, 3, 4, 5, 6, 7, 8, 9, 9, 9, 12, 13, 14, 16, 17, 18, 19, 20, 21]

BG, VG = 2, 4       # batch groups x vertex groups
BSH = B // BG       # 256
T2 = BSH // 128     # 2 partition tiles per core
NVP = 6912          # padded vertex count (divisible by VG and by 256-chunks)
NSH = NVP // VG     # 1728 vertices per core
NC3 = NSH * 3       # 5184 coordinate columns per core
CH = 432            # mm1/mm2 chunk width (1 PSUM bank, 12 chunks)
NCHUNKS = NC3 // CH
TCH = 256           # T-matmul / combine chunk width (2 planes share a bank)

# chain levels: (child_start, n_children, parent_start, parent_is_broadcast)
CHAIN_GROUPS = [
    (1, 3, 0, True),
    (4, 3, 1, False),
    (7, 3, 4, False),
    (10, 3, 7, False),
    (13, 2, 9, True),
    (15, 3, 12, False),
    (18, 2, 16, False),
    (20, 2, 18, False),
    (22, 2, 20, False),
]


def _ins_bcast(v, pos, n):
    """Insert a step-0 (broadcast) dim at position `pos` (0 = partition dim)."""
    return bass.AP(tensor=v.tensor, offset=v.offset,
                   ap=list(v.ap[:pos]) + [[0, n]] + list(v.ap[pos:]))


def _app_bcast(v, n):
    """Append a trailing step-0 dim."""
    return bass.AP(tensor=v.tensor, offset=v.offset, ap=list(v.ap) + [[0, n]])


def _build_program():
    import math
    from contextlib import ExitStack

    nc = bacc.Bacc("TRN2", target_bir_lowering=False, debug=False)

    rhs1 = nc.dram_tensor("rhs1", [KA, NC3 + 144], F32, kind="ExternalInput")
    pdT = nc.dram_tensor("pdT", [F, NC3], F32, kind="ExternalInput")
    wT = nc.dram_tensor("wT", [NJ, NSH], F32, kind="ExternalInput")
    poseil = nc.dram_tensor("poseil", [128, T2 * 72], F32, kind="ExternalInput")
    transil = nc.dram_tensor("transil", [128, T2 * 3], F32, kind="ExternalInput")
    betasT = nc.dram_tensor("betasT", [KA, BSH], F32, kind="ExternalInput")
    b1 = nc.dram_tensor("b1", [T2, 128], F32, kind="ExternalInput")

    vs_out = nc.dram_tensor("vs_out", [BSH, NC3], F32, kind="ExternalOutput")
    vp_out = nc.dram_tensor("vp_out", [BSH, NC3], F32, kind="ExternalOutput")
    v_out = nc.dram_tensor("v_out", [BSH, NC3], F32, kind="ExternalOutput")
    jt_out = nc.dram_tensor("jt_out", [BSH, 72], F32, kind="ExternalOutput")

    with tile.TileContext(nc) as tc, ExitStack() as ctx:
        const = ctx.enter_context(tc.tile_pool(name="const", bufs=1))
        big = ctx.enter_context(tc.tile_pool(name="big", bufs=1))
        rhsp = ctx.enter_context(tc.tile_pool(name="rhsp", bufs=3))
        sb = ctx.enter_context(tc.tile_pool(name="sb", bufs=3))
        qtmp = ctx.enter_context(tc.tile_pool(name="qtmp", bufs=2))
        # PSUM budget (8 banks): mm1 x2, mm2 x1, transposes x1, T-matmul 2x2
        psA = ctx.enter_context(tc.tile_pool(name="psA", bufs=2, space="PSUM"))
        psX = ctx.enter_context(tc.tile_pool(name="psX", bufs=1, space="PSUM"))
        psT = ctx.enter_context(tc.tile_pool(name="psT", bufs=2, space="PSUM"))

        # ---------------- constants / small loads ----------------
        ident = const.tile([128, 128], F32)
        make_identity(nc, ident)

        wT_sb = const.tile([NJ, NSH], F32)
        nc.sync.dma_start(wT_sb[:], wT[:])

        pose_sb = const.tile([128, T2 * 72], F32)
        nc.sync.dma_start(pose_sb[:], poseil[:])
        trans_sb = const.tile([128, T2 * 3], F32)
        nc.sync.dma_start(trans_sb[:], transil[:])

        bT_sb = []
        for k in range(3):
            ks = min(128, KA - k * 128)
            t_ = const.tile([128, BSH], F32, tag=f"bT{k}")
            nc.sync.dma_start(t_[:ks, :], betasT[k * 128:k * 128 + ks, :])
            bT_sb.append((t_, ks))

        # ---------------- quaternions / rotations ----------------
        # layouts: pose_sb [128,(t,j,3)], Q [128,(t,j,4)], R [128,(t,j,9)]
        NTJ = T2 * NJ
        pv = pose_sb[:].rearrange("p (g c) -> p g c", c=3)          # [128,48,3]
        tp = qtmp.tile([128, NTJ, 3], F32, tag="tp")
        nc.vector.tensor_scalar_add(tp[:], pv, 1e-8)
        sq = qtmp.tile([128, NTJ, 3], F32, tag="sq")
        nc.vector.tensor_mul(sq[:], tp[:], tp[:])
        a2 = qtmp.tile([128, NTJ], F32, tag="a2")
        nc.vector.tensor_add(a2[:], sq[:, :, 0], sq[:, :, 1])
        nc.vector.tensor_add(a2[:], a2[:], sq[:, :, 2])
        ang = qtmp.tile([128, NTJ], F32, tag="ang")
        nc.scalar.sqrt(ang[:], a2[:])
        inva = qtmp.tile([128, NTJ], F32, tag="inva")
        nc.vector.reciprocal(inva[:], ang[:])
        halfpi = const.tile([128, 1], F32)
        nc.vector.memset(halfpi[:], math.pi / 2)
        zerob = const.tile([128, 1], F32)
        nc.vector.memset(zerob[:], 0.0)
        s_ = qtmp.tile([128, NTJ], F32, tag="s_")
        nc.scalar.activation(s_[:], ang[:], mybir.ActivationFunctionType.Sin,
                             scale=0.5, bias=zerob[:])
        c_ = qtmp.tile([128, NTJ], F32, tag="c_")
        nc.scalar.activation(c_[:], ang[:], mybir.ActivationFunctionType.Sin,
                             scale=0.5, bias=halfpi[:])
        sn = qtmp.tile([128, NTJ], F32, tag="sn")
        nc.vector.tensor_mul(sn[:], s_[:], inva[:])

        Q_sb = big.tile([128, T2 * NJ * 4], F32, tag="Q")
        Qv = Q_sb[:].rearrange("p (g f) -> p g f", f=4)             # [128,48,4]
        for cc in range(3):
            nc.vector.tensor_mul(Qv[:, :, cc], sn[:], pv[:, :, cc])
        nc.vector.tensor_scalar_add(Qv[:, :, 3], c_[:], -1.0)

        qq = qtmp.tile([128, NTJ], F32, tag="qq")
        nc.vector.tensor_mul(qq[:], c_[:], c_[:])
        tq = qtmp.tile([128, NTJ], F32, tag="tq")
        for cc in range(3):
            nc.vector.tensor_mul(tq[:], Qv[:, :, cc], Qv[:, :, cc])
            nc.vector.tensor_add(qq[:], qq[:], tq[:])
        nc.scalar.sqrt(tq[:], qq[:])
        rq = qtmp.tile([128, NTJ], F32, tag="rq")
        nc.vector.reciprocal(rq[:], tq[:])

        wn = qtmp.tile([128, NTJ], F32, tag="wn")
        xn = qtmp.tile([128, NTJ], F32, tag="xn")
        yn = qtmp.tile([128, NTJ], F32, tag="yn")
        zn = qtmp.tile([128, NTJ], F32, tag="zn")
        nc.vector.tensor_mul(wn[:], c_[:], rq[:])
        nc.vector.tensor_mul(xn[:], Qv[:, :, 0], rq[:])
        nc.vector.tensor_mul(yn[:], Qv[:, :, 1], rq[:])
        nc.vector.tensor_mul(zn[:], Qv[:, :, 2], rq[:])

        pr = {}
        for nm, (u, v_) in {
            "w2": (wn, wn), "x2": (xn, xn), "y2": (yn, yn), "z2": (zn, zn),
            "xy": (xn, yn), "xz": (xn, zn), "yz": (yn, zn),
            "wx": (wn, xn), "wy": (wn, yn), "wz": (wn, zn),
        }.items():
            t_ = qtmp.tile([128, NTJ], F32, tag=nm)
            nc.vector.tensor_mul(t_[:], u[:], v_[:])
            pr[nm] = t_

        R_sb = big.tile([128, T2 * NJ * 9], F32, tag="R")
        Rv = R_sb[:].rearrange("p (g e) -> p g e", e=9)              # [128,48,9]
        u1 = qtmp.tile([128, NTJ], F32, tag="u1")
        u2 = qtmp.tile([128, NTJ], F32, tag="u2")
        # diagonals
        nc.vector.tensor_add(u1[:], pr["w2"][:], pr["x2"][:])
        nc.vector.tensor_add(u2[:], pr["y2"][:], pr["z2"][:])
        nc.vector.tensor_sub(Rv[:, :, 0], u1[:], u2[:])
        nc.vector.tensor_add(u1[:], pr["w2"][:], pr["y2"][:])
        nc.vector.tensor_add(u2[:], pr["x2"][:], pr["z2"][:])
        nc.vector.tensor_sub(Rv[:, :, 4], u1[:], u2[:])
        nc.vector.tensor_add(u1[:], pr["w2"][:], pr["z2"][:])
        nc.vector.tensor_add(u2[:], pr["x2"][:], pr["y2"][:])
        nc.vector.tensor_sub(Rv[:, :, 8], u1[:], u2[:])
        # off-diagonals (2*(a +- b))
        for e, an, bn, sub in ((1, "xy", "wz", True), (2, "wy", "xz", False),
                               (3, "wz", "xy", False), (5, "yz", "wx", True),
                               (6, "xz", "wy", True), (7, "wx", "yz", False)):
            if sub:
                nc.vector.tensor_sub(u1[:], pr[an][:], pr[bn][:])
            else:
                nc.vector.tensor_add(u1[:], pr[an][:], pr[bn][:])
            nc.vector.tensor_scalar_mul(Rv[:, :, e], u1[:], 2.0)

        # ---------------- pose_feat^T via PE transpose ----------------
        featT = []
        for t in range(T2):
            psq = psX.tile([128, 128], F32, tag="pst")
            nc.tensor.transpose(psq[:92, :], Q_sb[:, t * 96 + 4:(t + 1) * 96], ident[:])
            ft = big.tile([F, 128], F32, tag=f"ft{t}")
            nc.scalar.copy(ft[0:92, :], psq[0:92, :])
            nc.sync.dma_start(ft[92:93, :], b1[t:t + 1, :])
            featT.append(ft)

        # ---------------- mm1 (+J,+LC) and mm2 ----------------
        vp_full = [big.tile([128, NC3], F32, tag=f"vp{t}", name=f"vp{t}")
                   for t in range(T2)]
        J_all = big.tile([128, T2 * 72], F32, tag="J")
        LC_all = big.tile([128, T2 * 72], F32, tag="LC")

        for t in range(T2):
            for ch in range(NCHUNKS):
                cs = ch * CH
                rcs = []
                for k in range(3):
                    ks = bT_sb[k][1]
                    rc_ = rhsp.tile([128, CH], F32, tag=f"r1_{k}")
                    nc.sync.dma_start(rc_[:ks, :], rhs1[k * 128:k * 128 + ks, cs:cs + CH])
                    rcs.append(rc_)
                pdc = rhsp.tile([F, CH], F32, tag="pd")
                nc.sync.dma_start(pdc[:], pdT[:, cs:cs + CH])

                ps = psA.tile([128, CH], F32, tag="mm1")
                for k in range(3):
                    ks = bT_sb[k][1]
                    nc.tensor.matmul(ps[:], bT_sb[k][0][:ks, t * 128:(t + 1) * 128],
                                     rcs[k][:ks, :], start=(k == 0), stop=(k == 2))
                vs_sb = sb.tile([128, CH], F32, tag="vs")
                nc.scalar.copy(vs_sb[:], ps[:])
                nc.sync.dma_start(vs_out[t * 128:(t + 1) * 128, cs:cs + CH], vs_sb[:])

                ps2 = psA.tile([128, CH], F32, tag="mm2", bufs=1)
                nc.tensor.matmul(ps2[:], featT[t][:], pdc[:], start=True, stop=True)
                nc.vector.tensor_add(vp_full[t][:, cs:cs + CH], ps2[:], vs_sb[:])
                nc.sync.dma_start(vp_out[t * 128:(t + 1) * 128, cs:cs + CH],
                                  vp_full[t][:, cs:cs + CH])

            # J / LC columns
            psj = psA.tile([128, 432], F32, tag="mm1")
            for k in range(3):
                ks = bT_sb[k][1]
                rc_ = rhsp.tile([128, 144], F32, tag=f"rj_{k}")
                nc.sync.dma_start(rc_[:ks, :], rhs1[k * 128:k * 128 + ks, NC3:NC3 + 144])
                nc.tensor.matmul(psj[:, 0:144], bT_sb[k][0][:ks, t * 128:(t + 1) * 128],
                                 rc_[:ks, :], start=(k == 0), stop=(k == 2))
            nc.scalar.copy(J_all[:, t * 72:(t + 1) * 72], psj[:, 0:72])
            nc.scalar.copy(LC_all[:, t * 72:(t + 1) * 72], psj[:, 72:144])

        # ---------------- kinematic chain ----------------
        # G layout [128, (t, r3, c4, j32)] ; free stride: t=384, r=128, c=32, j=1
        G_sb = big.tile([128, T2 * 384], F32, tag="G")
        Gr = G_sb[:].rearrange("p (t r c j) -> p t r c j", t=T2, r=3, c=4, j=32)
        Rr = R_sb[:].rearrange("p (t j e) -> p t j e", t=T2, j=NJ, e=9)
        LCr = LC_all[:].rearrange("p (t j c) -> p t j c", t=T2, j=NJ, c=3)
        Jr = J_all[:].rearrange("p (t j c) -> p t j c", t=T2, j=NJ, c=3)
        trv = trans_sb[:].rearrange("p (t c) -> p t c", c=3)

        # root: rotation + translation (+trans fold)
        nc.vector.tensor_copy(Gr[:, :, :, 0:3, 0],
                              Rr[:, :, 0, :].rearrange("p t (r c) -> p t r c", c=3))
        nc.vector.tensor_add(Gr[:, :, :, 3, 0], LCr[:, :, 0, :].rearrange("p t c -> p t c"),
                             trv)

        for (ja, jn, pa, isb) in CHAIN_GROUPS:
            jsl = slice(ja, ja + jn)
            for t in range(T2):

                def pview(k):
                    if isb:
                        return _app_bcast(Gr[:, t, :, k, pa], jn)       # [128,3,jn]
                    return Gr[:, t, :, k, pa:pa + jn]

                # big group: c<3 -> out[r,c,j]
                outb = Gr[:, t, :, 0:3, jsl]
                tmp = sb.tile([128, 3, 3, jn], F32, tag="chtmp")
                tmp2 = sb.tile([128, 3, 3, jn], F32, tag="chtmp2")
                for k in range(3):
                    ck = Rr[:, t, jsl, 3 * k:3 * k + 3].rearrange("p j c -> p c j")
                    ckb = _ins_bcast(ck, 1, 3)                          # r bcast
                    pkb = _ins_bcast(pview(k), 2, 3)                    # c bcast
                    if k == 0:
                        nc.vector.tensor_mul(tmp[:], pkb, ckb)
                    else:
                        nc.vector.tensor_mul(tmp2[:], pkb, ckb)
                        nc.vector.tensor_add(tmp[:] if k == 1 else outb,
                                             tmp[:], tmp2[:])
                # small group: c=3 -> out[r,j]
                out3 = Gr[:, t, :, 3, jsl]
                u_ = sb.tile([128, 3, jn], F32, tag="chu")
                u2_ = sb.tile([128, 3, jn], F32, tag="chu2")
                for k in range(3):
                    c3k = _ins_bcast(LCr[:, t, jsl, k], 1, 3)           # r bcast
                    if k == 0:
                        nc.vector.tensor_mul(u_[:], pview(0), c3k)
                    else:
                        nc.vector.tensor_mul(u2_[:], pview(k), c3k)
                        nc.vector.tensor_add(u_[:], u_[:], u2_[:])
                nc.vector.tensor_add(out3, u_[:], pview(3))

        # ---------------- J_transformed (pre-correction chain translations) ----------------
        jt_sb = big.tile([128, T2 * 72], F32, tag="jt")
        nc.vector.tensor_copy(
            jt_sb[:].rearrange("p (t j r) -> p t j r", t=T2, j=NJ, r=3),
            Gr[:, :, :, 3, 0:NJ].rearrange("p t r j -> p t j r"))
        nc.sync.dma_start(jt_out[:].rearrange("(t p) f -> p t f", p=128),
                          jt_sb[:].rearrange("p (t f) -> p t f", f=72))

        # ---------------- last-column fix: t_G -= R_G @ J_rest ----------------
        fx = sb.tile([128, T2, 3, NJ], F32, tag="fx")
        fx2 = sb.tile([128, T2, 3, NJ], F32, tag="fx2")
        for cc in range(3):
            jb = _ins_bcast(Jr[:, :, :, cc], 2, 3)                      # [128,t,3b,24]
            if cc == 0:
                nc.vector.tensor_mul(fx[:], Gr[:, :, :, 0, 0:NJ], jb)
            else:
                nc.vector.tensor_mul(fx2[:], Gr[:, :, :, cc, 0:NJ], jb)
                nc.vector.tensor_add(fx[:], fx[:], fx2[:])
        nc.vector.tensor_sub(Gr[:, :, :, 3, 0:NJ], Gr[:, :, :, 3, 0:NJ], fx[:])

        # ---------------- transpose G -> GT tiles (one [24,128] per rc) ----------------
        GT = [[None] * 12 for _ in range(T2)]
        for t in range(T2):
            for rc in range(12):
                r, c = rc // 4, rc % 4
                pst = psX.tile([NJ, 128], F32, tag="pst")
                src = G_sb[:, t * 384 + r * 128 + c * 32:t * 384 + r * 128 + c * 32 + NJ]
                nc.tensor.transpose(pst[:], src, ident[:])
                gt = big.tile([NJ, 128], F32, tag=f"gt{t}_{rc}", name=f"gt{t}_{rc}")
                nc.scalar.copy(gt[:], pst[:])
                GT[t][rc] = gt

        # ---------------- T-matmul + LBS combine ----------------
        nt_chunks = (NSH + TCH - 1) // TCH
        for t in range(T2):
            for nch in range(nt_chunks):
                ns = nch * TCH
                sz = min(TCH, NSH - ns)
                vsb = sb.tile([128, TCH * 3], F32, tag="vstage")
                for r in range(3):
                    pa_ = psT.tile([128, 512], F32, tag="TA")
                    pb_ = psT.tile([128, 512], F32, tag="TB")
                    for c in range(4):
                        dst = (pa_ if c < 2 else pb_)
                        off = (c % 2) * 256
                        nc.tensor.matmul(dst[:, off:off + sz],
                                         GT[t][r * 4 + c][:],
                                         wT_sb[:, ns:ns + sz],
                                         start=True, stop=True)
                    xv = vp_full[t][:, ns * 3 + 0:(ns + sz) * 3:3]
                    yv = vp_full[t][:, ns * 3 + 1:(ns + sz) * 3:3]
                    zv = vp_full[t][:, ns * 3 + 2:(ns + sz) * 3:3]
                    ta = sb.tile([128, TCH], F32, tag="cta")
                    tb = sb.tile([128, TCH], F32, tag="ctb")
                    nc.vector.tensor_mul(ta[:, :sz], pa_[:, 0:sz], xv)
                    nc.vector.tensor_mul(tb[:, :sz], pa_[:, 256:256 + sz], yv)
                    nc.vector.tensor_add(ta[:, :sz], ta[:, :sz], tb[:, :sz])
                    nc.vector.tensor_mul(tb[:, :sz], pb_[:, 0:sz], zv)
                    nc.vector.tensor_add(ta[:, :sz], ta[:, :sz], tb[:, :sz])
                    nc.vector.tensor_add(vsb[:, r:sz * 3:3], ta[:, :sz],
                                         pb_[:, 256:256 + sz])
                nc.sync.dma_start(v_out[t * 128:(t + 1) * 128, ns * 3:(ns + sz) * 3],
                                  vsb[:, :sz * 3])

    nc.compile()
    return nc


_NC = None


def _get_program():
    global _NC
    if _NC is None:
        _NC = _build_program()
    return _NC


def kernel(pose, betas, trans, v_template, shapedirs, posedirs, J_regressor, weights):
    pose = np.asarray(pose, np.float32)
    betas = np.asarray(betas, np.float32)
    trans = np.asarray(trans, np.float32)
    v_template = np.asarray(v_template, np.float32)
    shapedirs = np.asarray(shapedirs, np.float32)
    posedirs = np.asarray(posedirs, np.float32)
    J_regressor = np.asarray(J_regressor, np.float32)
    weights = np.asarray(weights, np.float32)

    # ----- host precompute -----
    sd_flat = shapedirs.reshape(NV * 3, K)
    sdT_aug = np.zeros((KA, NVP * 3), np.float32)
    sdT_aug[:K, :NV * 3] = sd_flat.T
    sdT_aug[K, :NV * 3] = v_template.reshape(-1)

    js = np.einsum('jn,nck->kjc', J_regressor, shapedirs).reshape(K, 72)
    jvt = (J_regressor @ v_template).reshape(72)
    JS_aug = np.concatenate([js, jvt[None, :]], 0).astype(np.float32)       # [301,72]
    JS4 = JS_aug.reshape(KA, NJ, 3)
    JSL4 = JS4.copy()
    for j in range(1, NJ):
        JSL4[:, j, :] = JS4[:, j, :] - JS4[:, PARENT[j - 1], :]
    JSL_aug = JSL4.reshape(KA, 72)

    pdT_pad = np.zeros((F, NVP * 3), np.float32)
    pdT_pad[:, :NV * 3] = posedirs.T
    wT_pad = np.zeros((NJ, NVP), np.float32)
    wT_pad[:, :NV] = weights.T

    in_maps = []
    for core in range(8):
        bx, vx = core // VG, core % VG
        bsl = slice(bx * BSH, (bx + 1) * BSH)
        csl = slice(vx * NC3, (vx + 1) * NC3)
        vsl = slice(vx * NSH, (vx + 1) * NSH)
        betas_b = betas[bsl]
        in_maps.append({
            "rhs1": np.ascontiguousarray(
                np.concatenate([sdT_aug[:, csl], JS_aug, JSL_aug], 1)),
            "pdT": np.ascontiguousarray(pdT_pad[:, csl]),
            "wT": np.ascontiguousarray(wT_pad[:, vsl]),
            "poseil": np.ascontiguousarray(
                pose[bsl].reshape(T2, 128, 72).transpose(1, 0, 2).reshape(128, T2 * 72)),
            "transil": np.ascontiguousarray(
                trans[bsl].reshape(T2, 128, 3).transpose(1, 0, 2).reshape(128, T2 * 3)),
            "betasT": np.ascontiguousarray(
                np.concatenate([betas_b.T, np.ones((1, BSH), np.float32)], 0)),
            "b1": np.ascontiguousarray(betas_b[:, 1].reshape(T2, 128)),
        })

    global _last_in_maps
    _last_in_maps = in_maps
    nc = _get_program()
    res = bass_utils.run_bass_kernel_spmd(nc, in_maps, core_ids=list(range(8))).results

    vsf = np.empty((B, NVP * 3), np.float32)
    vpf = np.empty((B, NVP * 3), np.float32)
    vf = np.empty((B, NVP * 3), np.float32)
    jt = np.empty((B, NJ, 3), np.float32)
    for core in range(8):
        bx, vx = core // VG, core % VG
        bsl = slice(bx * BSH, (bx + 1) * BSH)
        csl = slice(vx * NC3, (vx + 1) * NC3)
        vsf[bsl, csl] = res[core]["vs_out"]
        vpf[bsl, csl] = res[core]["vp_out"]
        vf[bsl, csl] = res[core]["v_out"]
        if vx == 0:
            jt[bsl] = res[core]["jt_out"].reshape(BSH, NJ, 3)

    v = vf[:, :NV * 3].reshape(B, NV, 3)
    v_posed = vpf[:, :NV * 3].reshape(B, NV, 3)
    v_shaped = vsf[:, :NV * 3].reshape(B, NV, 3)
    return v, v_posed, v_shaped, jt
